# revision 12
# baseline (speedup 1.0000x reference)
"""Trainium2 Bass kernel for a 2-layer tanh RNN.

Model (per timestep t):
    u_t   = x_t @ W_in + b_in                                   [B, H]
    h0_t  = tanh(u_t @ W_i2h[0] + b_i2h[0] + h0_{t-1} @ W_h2h[0] + b_h2h[0])
    h1_t  = tanh(h0_t @ W_i2h[1] + b_i2h[1] + h1_{t-1} @ W_h2h[1] + b_h2h[1])
    y_t   = h1_t @ W_out + b_out                                [B, O]

Strategy: data-parallel over batch (8 cores x 8 rows).  Per core, all
input-path matmuls are hoisted out of the time loop into big batched
matmuls over all timesteps (A0 = (x@W_in+b_in)@W_i2h0 + biases, and after
the layer-0 recurrence, A1 = h0@W_i2h1 + biases).  Only h @ W_h2h remains
sequential.  The hidden state is kept transposed [H, B] (H on SBUF
partitions) so each step's matmuls are layout-preserving: no on-device
transposes anywhere; all transposes happen host-side in numpy.
"""

import os

import numpy as np

import concourse.bass as bass
from concourse import bacc
import concourse.mybir as mybir
import concourse.tile as tile
from concourse.bass_utils import run_bass_kernel_spmd

# Problem sizes (hardcoded per harness contract)
I, H, O, L = 256, 512, 256, 2
B, S = 64, 512
NCORES = 8
BC = B // NCORES  # batch rows per core
RC = 256  # row-chunk (t*BC+b rows) for the batched phases

F32 = mybir.dt.float32

# Matmul operand dtype knob: "f32", "f32r", "bf16", "f16"
MM_DTYPE = os.environ.get("KERNEL_MM_DTYPE", "f32r")
_DT = {
    "f32": mybir.dt.float32,
    "f32r": mybir.dt.float32r,
    "bf16": mybir.dt.bfloat16,
    "f16": mybir.dt.float16,
}[MM_DTYPE]

last_exec_time_ns = None
last_results = None


def _np_dt(dt):
    return mybir.dt.np(dt)


def build_bass(S_=S, BC_=BC):
    """Build the single-core SPMD program (same NEFF on all 8 cores)."""
    nc = bacc.Bacc("TRN2", target_bir_lowering=False, debug=False,
                   num_devices=NCORES)
    ROWS = S_ * BC_  # batched-phase row count per core
    NRC = ROWS // RC if ROWS >= RC else 1
    rc = min(RC, ROWS)

    # ---- I/O -------------------------------------------------------------
    xT = nc.dram_tensor("xT", [I, ROWS], _DT, kind="ExternalInput")
    hprevT = nc.dram_tensor("hprevT", [L, 128, 4 * BC_], _DT, kind="ExternalInput")
    w_in = nc.dram_tensor("w_in", [I, H], _DT, kind="ExternalInput")
    w0i = nc.dram_tensor("w0i", [H, H], _DT, kind="ExternalInput")
    w0h = nc.dram_tensor("w0h", [H, H], _DT, kind="ExternalInput")
    w1i = nc.dram_tensor("w1i", [H, H], _DT, kind="ExternalInput")
    w1h = nc.dram_tensor("w1h", [H, H], _DT, kind="ExternalInput")
    w_out = nc.dram_tensor("w_out", [H, O], _DT, kind="ExternalInput")
    b_in = nc.dram_tensor("b_in", [128, 4], F32, kind="ExternalInput")
    bias0 = nc.dram_tensor("bias0", [128, 4], F32, kind="ExternalInput")
    bias1 = nc.dram_tensor("bias1", [128, 4], F32, kind="ExternalInput")
    b_out = nc.dram_tensor("b_out", [128, 2], F32, kind="ExternalInput")

    yT = nc.dram_tensor("yT", [O, ROWS], F32, kind="ExternalOutput")
    h_final = nc.dram_tensor("h_final", [L, 128, 4 * BC_], F32, kind="ExternalOutput")

    with tile.TileContext(nc) as tc:
        with (
            tc.tile_pool(name="consts", bufs=1) as consts,
            tc.tile_pool(name="big", bufs=1) as bigpool,
        ):
            # ---- load weights/biases into SBUF ---------------------------
            # lhsT chunk layout: [K=128 partitions, chunk, M<=128]
            win_sb = consts.tile([128, 2, H], _DT)
            nc.sync.dma_start(
                out=win_sb, in_=w_in.rearrange("(k p) n -> p k n", p=128)
            )
            w_sb = {}
            for name, dram in (("w0i", w0i), ("w0h", w0h), ("w1i", w1i), ("w1h", w1h)):
                t = consts.tile([128, 16, 128], _DT, tag=name)
                nc.sync.dma_start(
                    out=t.rearrange("p (k n) c -> p k n c", k=4),
                    in_=dram.rearrange("(k p) (n c) -> p k n c", p=128, n=4),
                )
                w_sb[name] = t
            wout_sb = consts.tile([128, 8, 128], _DT)
            nc.sync.dma_start(
                out=wout_sb.rearrange("p (k m) c -> p k m c", k=4),
                in_=w_out.rearrange("(k p) (m c) -> p k m c", p=128, m=2),
            )
            b_in_sb = consts.tile([128, 4], F32, tag="b_in")
            nc.sync.dma_start(out=b_in_sb, in_=b_in[:, :])
            bias_sb = {}
            for name, dram in (("bias0", bias0), ("bias1", bias1)):
                t = consts.tile([128, 4], F32, tag=name)
                nc.sync.dma_start(out=t, in_=dram[:, :])
                bias_sb[name] = t
            b_out_sb = consts.tile([128, 2], F32, tag="b_out")
            nc.sync.dma_start(out=b_out_sb, in_=b_out[:, :])

            # ---- big persistent buffers ----------------------------------
            # A: precomputed input-path pre-activations, [H -> (4,128)], t-major
            A_sb = bigpool.tile([128, 4, ROWS], F32)
            # hidden-state archive: slot s = h_{s-1}; slot 0 = h_prev
            harch = bigpool.tile([128, 4, (S_ + 1) * BC_], _DT)

            # ================ Phase A0 ====================================
            with (
                nc.named_scope("phase_a0"),
                tc.tile_pool(name="xr", bufs=2) as xpool,
                tc.tile_pool(name="u", bufs=2) as upool,
                tc.tile_pool(name="pp", bufs=8, space="PSUM") as ppool,
            ):
                for r in range(NRC):
                    xr = xpool.tile([128, 2, rc], _DT, tag="xr")
                    nc.sync.dma_start(
                        out=xr,
                        in_=xT[:, r * rc:(r + 1) * rc].rearrange(
                            "(k p) r -> p k r", p=128
                        ),
                    )
                    u = upool.tile([128, 4, rc], _DT, tag="u")
                    for m in range(4):
                        pu = ppool.tile([128, rc], F32, tag="pp")
                        for k in range(2):
                            nc.tensor.matmul(
                                pu[:, :],
                                win_sb[:, k, m * 128:(m + 1) * 128],
                                xr[:, k, :],
                                start=(k == 0),
                                stop=(k == 1),
                            )
                        nc.scalar.activation(
                            u[:, m, :], pu[:, :],
                            mybir.ActivationFunctionType.Identity,
                            bias=b_in_sb[:, m:m + 1],
                        )
                    for n in range(4):
                        pa = ppool.tile([128, rc], F32, tag="pp")
                        for k in range(4):
                            nc.tensor.matmul(
                                pa[:, :],
                                w_sb["w0i"][:, k * 4 + n, :],
                                u[:, k, :],
                                start=(k == 0),
                                stop=(k == 3),
                            )
                        nc.scalar.activation(
                            A_sb[:, n, r * rc:(r + 1) * rc], pa[:, :],
                            mybir.ActivationFunctionType.Identity,
                            bias=bias_sb["bias0"][:, n:n + 1],
                        )

            # ================ Recurrence L0 ===============================
            nc.sync.dma_start(
                out=harch[:, :, 0:BC_],
                in_=hprevT[0].rearrange("p (n b) -> p n b", n=4),
            )
            with (
                nc.named_scope("rec_l0"),
                tc.tile_pool(name="rp", bufs=4, space="PSUM") as rpool,
            ):
                _recurrence(nc, tc, rpool, harch, A_sb, w_sb["w0h"], S_, BC_)
            nc.gpsimd.dma_start(
                out=h_final[0].rearrange("p (n b) -> p n b", n=4),
                in_=harch[:, :, S_ * BC_:(S_ + 1) * BC_],
            )

            # ================ Phase A1 ====================================
            with (
                nc.named_scope("phase_a1"),
                tc.tile_pool(name="pp2", bufs=8, space="PSUM") as ppool,
            ):
                for r in range(NRC):
                    for n in range(4):
                        pa = ppool.tile([128, rc], F32, tag="pp2")
                        for k in range(4):
                            nc.tensor.matmul(
                                pa[:, :],
                                w_sb["w1i"][:, k * 4 + n, :],
                                harch[:, k, BC_ + r * rc:BC_ + (r + 1) * rc],
                                start=(k == 0),
                                stop=(k == 3),
                            )
                        nc.scalar.activation(
                            A_sb[:, n, r * rc:(r + 1) * rc], pa[:, :],
                            mybir.ActivationFunctionType.Identity,
                            bias=bias_sb["bias1"][:, n:n + 1],
                        )

            # ================ Recurrence L1 ===============================
            nc.sync.dma_start(
                out=harch[:, :, 0:BC_],
                in_=hprevT[1].rearrange("p (n b) -> p n b", n=4),
            )
            with (
                nc.named_scope("rec_l1"),
                tc.tile_pool(name="rp2", bufs=4, space="PSUM") as rpool,
            ):
                _recurrence(nc, tc, rpool, harch, A_sb, w_sb["w1h"], S_, BC_)
            nc.gpsimd.dma_start(
                out=h_final[1].rearrange("p (n b) -> p n b", n=4),
                in_=harch[:, :, S_ * BC_:(S_ + 1) * BC_],
            )

            # ================ Output phase ================================
            with (
                nc.named_scope("phase_out"),
                tc.tile_pool(name="ysb", bufs=2) as ypool,
                tc.tile_pool(name="pp3", bufs=8, space="PSUM") as ppool,
            ):
                for r in range(NRC):
                    y_sb = ypool.tile([128, 2, rc], F32, tag="ysb")
                    for m in range(2):
                        py = ppool.tile([128, rc], F32, tag="pp3")
                        for k in range(4):
                            nc.tensor.matmul(
                                py[:, :],
                                wout_sb[:, k * 2 + m, :],
                                harch[:, k, BC_ + r * rc:BC_ + (r + 1) * rc],
                                start=(k == 0),
                                stop=(k == 3),
                            )
                        nc.scalar.activation(
                            y_sb[:, m, :], py[:, :],
                            mybir.ActivationFunctionType.Identity,
                            bias=b_out_sb[:, m:m + 1],
                        )
                        nc.sync.dma_start(
                            out=yT[m * 128:(m + 1) * 128, r * rc:(r + 1) * rc],
                            in_=y_sb[:, m, :],
                        )

    nc.compile()
    return nc


def w_l_slice(dram, k):
    return dram[k * 128:(k + 1) * 128, :].rearrange("p (n c) -> p n c", n=4)


def _recurrence(nc, tc, rpool, harch, A_sb, w_sb, S_, BC_):
    for t in range(S_):
        ps = rpool.tile([128, 4 * BC_], F32, tag="rp")
        for n in range(4):
            for k in range(4):
                nc.tensor.matmul(
                    ps[:, n * BC_:(n + 1) * BC_],
                    w_sb[:, k * 4 + n, :],
                    harch[:, k, t * BC_:(t + 1) * BC_],
                    start=(k == 0),
                    stop=(k == 3),
                )
            nc.vector.tensor_add(
                ps[:, n * BC_:(n + 1) * BC_],
                ps[:, n * BC_:(n + 1) * BC_],
                A_sb[:, n, t * BC_:(t + 1) * BC_],
            )
            nc.scalar.activation(
                harch[:, n, (t + 1) * BC_:(t + 2) * BC_],
                ps[:, n * BC_:(n + 1) * BC_],
                mybir.ActivationFunctionType.Tanh,
            )


def _prep_inputs(x, h_prev, W_in, b_in, W_i2h, b_i2h, W_h2h, b_h2h, W_out, b_out,
                 S_=S, BC_=BC):
    """Host-side sharding + layout prep. Returns per-core in_maps."""
    np_dt = _np_dt(_DT)
    f32 = np.float32

    def cast(a):
        return np.ascontiguousarray(np.asarray(a, dtype=f32)).astype(np_dt)

    # shared tensors
    shared = {
        "w_in": cast(W_in),
        "w0i": cast(W_i2h[0]),
        "w0h": cast(W_h2h[0]),
        "w1i": cast(W_i2h[1]),
        "w1h": cast(W_h2h[1]),
        "w_out": cast(W_out),
        "b_in": np.asarray(b_in, f32).reshape(4, 128).T.copy(),
        "bias0": (np.asarray(b_i2h[0], f32) + np.asarray(b_h2h[0], f32)).reshape(4, 128).T.copy(),
        "bias1": (np.asarray(b_i2h[1], f32) + np.asarray(b_h2h[1], f32)).reshape(4, 128).T.copy(),
        "b_out": np.asarray(b_out, f32).reshape(2, 128).T.copy(),
    }
    in_maps = []
    x = np.asarray(x, f32)
    h_prev = np.asarray(h_prev, f32)
    for c in range(NCORES):
        xs = x[c * BC_:(c + 1) * BC_, :S_, :]  # [BC, S, I]
        xT = np.ascontiguousarray(xs.transpose(2, 1, 0).reshape(I, S_ * BC_)).astype(np_dt)
        hp = h_prev[:, c * BC_:(c + 1) * BC_, :]  # [L, BC, H]
        # [L, BC, H] -> [L, H, BC] -> [L, 4, 128, BC] -> [L, 128, 4, BC]
        hprevT = np.ascontiguousarray(
            hp.transpose(0, 2, 1).reshape(L, 4, 128, BC_).transpose(0, 2, 1, 3)
            .reshape(L, 128, 4 * BC_)
        ).astype(np_dt)
        in_maps.append({"xT": xT, "hprevT": hprevT, **shared})
    return in_maps


def _assemble_outputs(results, S_=S, BC_=BC):
    outs = np.empty((B, S_, O), np.float32)
    h_t = np.empty((L, B, H), np.float32)
    for c, res in enumerate(results):
        yT = res["yT"]  # [O, S*BC]
        outs[c * BC_:(c + 1) * BC_] = yT.reshape(O, S_, BC_).transpose(2, 1, 0)
        hf = res["h_final"].reshape(L, 128, 4, BC_)  # [L, 128p, 4n, BC]
        h_t[:, c * BC_:(c + 1) * BC_, :] = (
            hf.transpose(0, 3, 2, 1).reshape(L, BC_, H)
        )
    return outs, h_t


def _ensure_ntff_hook():
    """Register the axon NTFF profiling hook if the image's antenv lacks it."""
    import sys
    import types

    try:
        from antenv.axon_hooks import get_axon_ntff_profile_hook  # noqa: F401
        return
    except ImportError:
        pass
    import antenv
    from trn_agent_boot.trn_boot import _ntff_profile_via_ctypes

    hook = _ntff_profile_via_ctypes("/opt/axon/libaxon_pjrt.so")
    mod = types.ModuleType("antenv.axon_hooks")
    mod.get_axon_ntff_profile_hook = lambda: hook
    mod.set_axon_ntff_profile_hook = lambda h: None
    sys.modules["antenv.axon_hooks"] = mod
    antenv.axon_hooks = mod
    # no S3 in this container: keep artifacts local
    import concourse.bass_utils as bu

    bu.upload_artifacts = lambda tmpdir: tmpdir


def kernel(x, h_prev, W_in, b_in, W_i2h, b_i2h, W_h2h, b_h2h, W_out, b_out):
    global last_exec_time_ns, last_results
    nc = build_bass()
    in_maps = _prep_inputs(x, h_prev, W_in, b_in, W_i2h, b_i2h, W_h2h, b_h2h,
                           W_out, b_out)
    trace = os.environ.get("KERNEL_TRACE", "0") == "1"
    if trace:
        _ensure_ntff_hook()
    res = run_bass_kernel_spmd(nc, in_maps, core_ids=list(range(NCORES)),
                               trace=trace)
    last_exec_time_ns = res.exec_time_ns
    last_results = res
    return _assemble_outputs(res.results)


# revision 16
# speedup vs baseline: 2.9466x; 2.9466x over previous
"""Trainium2 Bass kernel for a 2-layer tanh RNN.

Model (per timestep t):
    u_t   = x_t @ W_in + b_in                                   [B, H]
    h0_t  = tanh(u_t @ W_i2h[0] + b_i2h[0] + h0_{t-1} @ W_h2h[0] + b_h2h[0])
    h1_t  = tanh(h0_t @ W_i2h[1] + b_i2h[1] + h1_{t-1} @ W_h2h[1] + b_h2h[1])
    y_t   = h1_t @ W_out + b_out                                [B, O]

Strategy: data-parallel over batch (8 cores x 8 rows).  Per core, all
input-path matmuls are hoisted out of the time loop into big batched
matmuls over all timesteps (A0 = (x@W_in+b_in)@W_i2h0 + biases, and after
the layer-0 recurrence, A1 = h0@W_i2h1 + biases).  Only h @ W_h2h remains
sequential.  The hidden state is kept transposed [H, B] (H on SBUF
partitions) so each step's matmuls are layout-preserving: no on-device
transposes anywhere; all transposes happen host-side in numpy.
"""

import os

import numpy as np

import concourse.bass as bass
from concourse import bacc
import concourse.mybir as mybir
import concourse.tile as tile
from concourse.bass_utils import run_bass_kernel_spmd

# Problem sizes (hardcoded per harness contract)
I, H, O, L = 256, 512, 256, 2
B, S = 64, 512
NCORES = 8
BC = B // NCORES  # batch rows per core
RC = 256  # row-chunk (t*BC+b rows) for the batched phases

F32 = mybir.dt.float32

# Matmul operand dtype knob: "f32", "f32r", "bf16", "f16"
MM_DTYPE = os.environ.get("KERNEL_MM_DTYPE", "f32r")
_DT = {
    "f32": mybir.dt.float32,
    "f32r": mybir.dt.float32r,
    "bf16": mybir.dt.bfloat16,
    "f16": mybir.dt.float16,
}[MM_DTYPE]

last_exec_time_ns = None
last_results = None


def _np_dt(dt):
    return mybir.dt.np(dt)


def build_bass(S_=S, BC_=BC):
    """Build the single-core SPMD program (same NEFF on all 8 cores)."""
    nc = bacc.Bacc("TRN2", target_bir_lowering=False, debug=False,
                   num_devices=NCORES)
    ROWS = S_ * BC_  # batched-phase row count per core
    NRC = ROWS // RC if ROWS >= RC else 1
    rc = min(RC, ROWS)

    # ---- I/O -------------------------------------------------------------
    xT = nc.dram_tensor("xT", [I, ROWS], _DT, kind="ExternalInput")
    hprevT = nc.dram_tensor("hprevT", [L, 128, 4 * BC_], _DT, kind="ExternalInput")
    w_in = nc.dram_tensor("w_in", [I, H], _DT, kind="ExternalInput")
    w0i = nc.dram_tensor("w0i", [H, H], _DT, kind="ExternalInput")
    w0h = nc.dram_tensor("w0h", [H, H], _DT, kind="ExternalInput")
    w1i = nc.dram_tensor("w1i", [H, H], _DT, kind="ExternalInput")
    w1h = nc.dram_tensor("w1h", [H, H], _DT, kind="ExternalInput")
    w_out = nc.dram_tensor("w_out", [H, O], _DT, kind="ExternalInput")
    b_in = nc.dram_tensor("b_in", [128, 4], F32, kind="ExternalInput")
    bias0 = nc.dram_tensor("bias0", [128, 4], F32, kind="ExternalInput")
    bias1 = nc.dram_tensor("bias1", [128, 4], F32, kind="ExternalInput")
    b_out = nc.dram_tensor("b_out", [128, 2], F32, kind="ExternalInput")
    ident_d = nc.dram_tensor("ident", [128, 128], _DT, kind="ExternalInput")

    yT = nc.dram_tensor("yT", [O, ROWS], F32, kind="ExternalOutput")
    h_final = nc.dram_tensor("h_final", [L, 128, 4 * BC_], F32, kind="ExternalOutput")

    with tile.TileContext(nc) as tc:
        with (
            tc.tile_pool(name="consts", bufs=1) as consts,
            tc.tile_pool(name="big", bufs=1) as bigpool,
        ):
            # ---- load weights/biases into SBUF ---------------------------
            # lhsT chunk layout: [K=128 partitions, chunk, M<=128]
            win_sb = consts.tile([128, 2, H], _DT)
            nc.sync.dma_start(
                out=win_sb, in_=w_in.rearrange("(k p) n -> p k n", p=128)
            )
            w_sb = {}
            for name, dram in (("w0i", w0i), ("w0h", w0h), ("w1i", w1i), ("w1h", w1h)):
                t = consts.tile([128, 16, 128], _DT, tag=name)
                nc.sync.dma_start(
                    out=t.rearrange("p (k n) c -> p k n c", k=4),
                    in_=dram.rearrange("(k p) (n c) -> p k n c", p=128, n=4),
                )
                w_sb[name] = t
            wout_sb = consts.tile([128, 8, 128], _DT)
            nc.sync.dma_start(
                out=wout_sb.rearrange("p (k m) c -> p k m c", k=4),
                in_=w_out.rearrange("(k p) (m c) -> p k m c", p=128, m=2),
            )
            b_in_sb = consts.tile([128, 4], F32, tag="b_in")
            nc.sync.dma_start(out=b_in_sb, in_=b_in[:, :])
            bias_sb = {}
            for name, dram in (("bias0", bias0), ("bias1", bias1)):
                t = consts.tile([128, 4], F32, tag=name)
                nc.sync.dma_start(out=t, in_=dram[:, :])
                bias_sb[name] = t
            b_out_sb = consts.tile([128, 2], F32, tag="b_out")
            nc.sync.dma_start(out=b_out_sb, in_=b_out[:, :])
            ident_sb = consts.tile([128, 128], _DT, tag="ident")
            nc.sync.dma_start(out=ident_sb, in_=ident_d[:, :])

            # ---- big persistent buffers ----------------------------------
            # A: precomputed input-path pre-activations, [H -> (4,128)], t-major
            A_sb = bigpool.tile([128, 4, ROWS], _DT)
            # hidden-state archive: slot s = h_{s-1}; slot 0 = h_prev
            harch = bigpool.tile([128, 4, (S_ + 1) * BC_], _DT)

            # ================ Phase A0 ====================================
            with (
                nc.named_scope("phase_a0"),
                tc.tile_pool(name="xr", bufs=2) as xpool,
                tc.tile_pool(name="u", bufs=2) as upool,
                tc.tile_pool(name="pp", bufs=8, space="PSUM") as ppool,
            ):
                for r in range(NRC):
                    xr = xpool.tile([128, 2, rc], _DT, tag="xr")
                    nc.sync.dma_start(
                        out=xr,
                        in_=xT[:, r * rc:(r + 1) * rc].rearrange(
                            "(k p) r -> p k r", p=128
                        ),
                    )
                    u = upool.tile([128, 4, rc], _DT, tag="u")
                    for m in range(4):
                        pu = ppool.tile([128, rc], F32, tag="pp")
                        for k in range(2):
                            nc.tensor.matmul(
                                pu[:, :],
                                win_sb[:, k, m * 128:(m + 1) * 128],
                                xr[:, k, :],
                                start=(k == 0),
                                stop=(k == 1),
                            )
                        nc.scalar.activation(
                            u[:, m, :], pu[:, :],
                            mybir.ActivationFunctionType.Identity,
                            bias=b_in_sb[:, m:m + 1],
                        )
                    for n in range(4):
                        pa = ppool.tile([128, rc], F32, tag="pp")
                        for k in range(4):
                            nc.tensor.matmul(
                                pa[:, :],
                                w_sb["w0i"][:, k * 4 + n, :],
                                u[:, k, :],
                                start=(k == 0),
                                stop=(k == 3),
                            )
                        nc.scalar.activation(
                            A_sb[:, n, r * rc:(r + 1) * rc], pa[:, :],
                            mybir.ActivationFunctionType.Identity,
                            bias=bias_sb["bias0"][:, n:n + 1],
                        )

            # ================ Recurrence L0 ===============================
            nc.sync.dma_start(
                out=harch[:, :, 0:BC_],
                in_=hprevT[0].rearrange("p (n b) -> p n b", n=4),
            )
            with (
                nc.named_scope("rec_l0"),
                tc.tile_pool(name="rp", bufs=4, space="PSUM") as rpool,
            ):
                _recurrence(nc, tc, rpool, harch, A_sb, w_sb["w0h"], ident_sb, S_, BC_)
            nc.gpsimd.dma_start(
                out=h_final[0].rearrange("p (n b) -> p n b", n=4),
                in_=harch[:, :, S_ * BC_:(S_ + 1) * BC_],
            )

            # ================ Phase A1 ====================================
            with (
                nc.named_scope("phase_a1"),
                tc.tile_pool(name="pp2", bufs=8, space="PSUM") as ppool,
            ):
                for r in range(NRC):
                    for n in range(4):
                        pa = ppool.tile([128, rc], F32, tag="pp2")
                        for k in range(4):
                            nc.tensor.matmul(
                                pa[:, :],
                                w_sb["w1i"][:, k * 4 + n, :],
                                harch[:, k, BC_ + r * rc:BC_ + (r + 1) * rc],
                                start=(k == 0),
                                stop=(k == 3),
                            )
                        nc.scalar.activation(
                            A_sb[:, n, r * rc:(r + 1) * rc], pa[:, :],
                            mybir.ActivationFunctionType.Identity,
                            bias=bias_sb["bias1"][:, n:n + 1],
                        )

            # ================ Recurrence L1 ===============================
            nc.sync.dma_start(
                out=harch[:, :, 0:BC_],
                in_=hprevT[1].rearrange("p (n b) -> p n b", n=4),
            )
            with (
                nc.named_scope("rec_l1"),
                tc.tile_pool(name="rp2", bufs=4, space="PSUM") as rpool,
            ):
                _recurrence(nc, tc, rpool, harch, A_sb, w_sb["w1h"], ident_sb, S_, BC_)
            nc.gpsimd.dma_start(
                out=h_final[1].rearrange("p (n b) -> p n b", n=4),
                in_=harch[:, :, S_ * BC_:(S_ + 1) * BC_],
            )

            # ================ Output phase ================================
            with (
                nc.named_scope("phase_out"),
                tc.tile_pool(name="ysb", bufs=2) as ypool,
                tc.tile_pool(name="pp3", bufs=8, space="PSUM") as ppool,
            ):
                for r in range(NRC):
                    y_sb = ypool.tile([128, 2, rc], F32, tag="ysb")
                    for m in range(2):
                        py = ppool.tile([128, rc], F32, tag="pp3")
                        for k in range(4):
                            nc.tensor.matmul(
                                py[:, :],
                                wout_sb[:, k * 2 + m, :],
                                harch[:, k, BC_ + r * rc:BC_ + (r + 1) * rc],
                                start=(k == 0),
                                stop=(k == 3),
                            )
                        nc.scalar.activation(
                            y_sb[:, m, :], py[:, :],
                            mybir.ActivationFunctionType.Identity,
                            bias=b_out_sb[:, m:m + 1],
                        )
                        nc.sync.dma_start(
                            out=yT[m * 128:(m + 1) * 128, r * rc:(r + 1) * rc],
                            in_=y_sb[:, m, :],
                        )

    nc.compile()
    return nc


def w_l_slice(dram, k):
    return dram[k * 128:(k + 1) * 128, :].rearrange("p (n c) -> p n c", n=4)


def _recurrence(nc, tc, rpool, harch, A_sb, w_sb, ident, S_, BC_):
    for t in range(S_):
        ps = rpool.tile([128, 4 * BC_], F32, tag="rp")
        for n in range(4):
            # group starter: psum[n] = I @ A_t[n]  (injects the precomputed
            # input-path pre-activation straight into the accumulator)
            nc.tensor.matmul(
                ps[:, n * BC_:(n + 1) * BC_],
                ident,
                A_sb[:, n, t * BC_:(t + 1) * BC_],
                start=True,
                stop=False,
            )
            for k in range(4):
                nc.tensor.matmul(
                    ps[:, n * BC_:(n + 1) * BC_],
                    w_sb[:, k * 4 + n, :],
                    harch[:, k, t * BC_:(t + 1) * BC_],
                    start=False,
                    stop=(k == 3),
                )
        nc.scalar.activation(
            harch[:, :, (t + 1) * BC_:(t + 2) * BC_],
            ps.rearrange("p (n b) -> p n b", n=4),
            mybir.ActivationFunctionType.Tanh,
        )


def _prep_inputs(x, h_prev, W_in, b_in, W_i2h, b_i2h, W_h2h, b_h2h, W_out, b_out,
                 S_=S, BC_=BC):
    """Host-side sharding + layout prep. Returns per-core in_maps."""
    np_dt = _np_dt(_DT)
    f32 = np.float32

    def cast(a):
        return np.ascontiguousarray(np.asarray(a, dtype=f32)).astype(np_dt)

    # shared tensors
    shared = {
        "w_in": cast(W_in),
        "w0i": cast(W_i2h[0]),
        "w0h": cast(W_h2h[0]),
        "w1i": cast(W_i2h[1]),
        "w1h": cast(W_h2h[1]),
        "w_out": cast(W_out),
        "b_in": np.asarray(b_in, f32).reshape(4, 128).T.copy(),
        "bias0": (np.asarray(b_i2h[0], f32) + np.asarray(b_h2h[0], f32)).reshape(4, 128).T.copy(),
        "bias1": (np.asarray(b_i2h[1], f32) + np.asarray(b_h2h[1], f32)).reshape(4, 128).T.copy(),
        "b_out": np.asarray(b_out, f32).reshape(2, 128).T.copy(),
        "ident": np.eye(128, dtype=f32).astype(np_dt),
    }
    in_maps = []
    x = np.asarray(x, f32)
    h_prev = np.asarray(h_prev, f32)
    for c in range(NCORES):
        xs = x[c * BC_:(c + 1) * BC_, :S_, :]  # [BC, S, I]
        xT = np.ascontiguousarray(xs.transpose(2, 1, 0).reshape(I, S_ * BC_)).astype(np_dt)
        hp = h_prev[:, c * BC_:(c + 1) * BC_, :]  # [L, BC, H]
        # [L, BC, H] -> [L, H, BC] -> [L, 4, 128, BC] -> [L, 128, 4, BC]
        hprevT = np.ascontiguousarray(
            hp.transpose(0, 2, 1).reshape(L, 4, 128, BC_).transpose(0, 2, 1, 3)
            .reshape(L, 128, 4 * BC_)
        ).astype(np_dt)
        in_maps.append({"xT": xT, "hprevT": hprevT, **shared})
    return in_maps


def _assemble_outputs(results, S_=S, BC_=BC):
    outs = np.empty((B, S_, O), np.float32)
    h_t = np.empty((L, B, H), np.float32)
    for c, res in enumerate(results):
        yT = res["yT"]  # [O, S*BC]
        outs[c * BC_:(c + 1) * BC_] = yT.reshape(O, S_, BC_).transpose(2, 1, 0)
        hf = res["h_final"].reshape(L, 128, 4, BC_)  # [L, 128p, 4n, BC]
        h_t[:, c * BC_:(c + 1) * BC_, :] = (
            hf.transpose(0, 3, 2, 1).reshape(L, BC_, H)
        )
    return outs, h_t


def _ensure_ntff_hook():
    """Register the axon NTFF profiling hook if the image's antenv lacks it."""
    import sys
    import types

    try:
        from antenv.axon_hooks import get_axon_ntff_profile_hook  # noqa: F401
        return
    except ImportError:
        pass
    import antenv
    from trn_agent_boot.trn_boot import _ntff_profile_via_ctypes

    hook = _ntff_profile_via_ctypes("/opt/axon/libaxon_pjrt.so")
    mod = types.ModuleType("antenv.axon_hooks")
    mod.get_axon_ntff_profile_hook = lambda: hook
    mod.set_axon_ntff_profile_hook = lambda h: None
    sys.modules["antenv.axon_hooks"] = mod
    antenv.axon_hooks = mod
    # no S3 in this container: keep artifacts local
    import concourse.bass_utils as bu

    bu.upload_artifacts = lambda tmpdir: tmpdir


def kernel(x, h_prev, W_in, b_in, W_i2h, b_i2h, W_h2h, b_h2h, W_out, b_out):
    global last_exec_time_ns, last_results
    nc = build_bass()
    in_maps = _prep_inputs(x, h_prev, W_in, b_in, W_i2h, b_i2h, W_h2h, b_h2h,
                           W_out, b_out)
    trace = os.environ.get("KERNEL_TRACE", "0") == "1"
    if trace:
        _ensure_ntff_hook()
    res = run_bass_kernel_spmd(nc, in_maps, core_ids=list(range(NCORES)),
                               trace=trace)
    last_exec_time_ns = res.exec_time_ns
    last_results = res
    return _assemble_outputs(res.results)
